# revision 8
# baseline (speedup 1.0000x reference)
"""Trainium2 Bass kernel for ContinuousREWAEncoder:
    out = FWHT(x @ W^T)/sqrt(32) + 0.01*normal(key=42)

Math folding: FWHT is linear => out = x @ (H @ W / sqrt(32))^T + noise.
The noise uses a fixed PRNG key => deterministic constant, added on HOST
(zero device cost, bit-identical to the reference noise).

Device math (per core, data parallel over tokens):
  x is streamed as fp8e4m3 (hi, lo) pairs:   x ~= xhi + xlo/16
  w is held as fp8 cells in a [128, 2, 64] DoubleRow stationary:
     out rows  0:32 cells (whi,    whi/16 ) -> psumA = whi*x
     out rows 32:64 cells (wlo/16, wlo/256) -> psumB = (wlo/16)*x
  where 16*w_eff ~= whi + wlo/16.  A DoubleRow matmul ingests both fp8
  planes in one pass, and psumA+psumB = 16*w_eff*x to ~1e-3 max rel err.
  DVE stages psumB into SBUF and adds psumA -> fp16 (only DVE/Act read
  PSUM, one PSUM operand per op; Act is avoided because its activation-
  table load stalls the scalar DMA queue).  The host divides by 16 and
  adds the noise.

Sharding: 4096 tokens/core on 8 cores.  x is pre-tiled on host into one
[128, 65536] byte plane per core; every DMA segment is one contiguous
run of >= 4 KiB per partition (small runs halve DMA throughput).  The
first segments are single blocks so the PE starts early; the middle is
fetched in 16 KiB paired runs; the last 512-token block arrives as two
256-token segments feeding column halves of one PSUM bank, so only a
narrow matmul + evac + small DMA chain trails the final byte.
"""

import math

import ml_dtypes
import numpy as np

import concourse.tile as tile
from concourse import bacc, mybir
from concourse.bass_utils import run_bass_kernel_spmd

B, N, D, M = 4, 8192, 1024, 32
NOISE_STD = 0.01
N_CORES = 8
TOK_TOTAL = B * N              # 32768
TOK = TOK_TOTAL // N_CORES     # 4096 tokens per core
BLK = 512                      # tokens per PSUM bank ([64, 512] fp32 = 1 bank)
NBLK = TOK // BLK              # 8
KC = D // 128                  # 8 contraction chunks of 128 dims

FP8 = mybir.dt.float8e4
NP8 = ml_dtypes.float8_e4m3    # == mybir.dt.np(mybir.dt.float8e4)
F32 = mybir.dt.float32
F16 = mybir.dt.float16
DR = mybir.MatmulPerfMode.DoubleRow

X_BYTES = TOK * D * 2 // 128   # 65536 fp8 bytes per partition per core

# stream segments: (blocks, token-range within last block) — encoded below.
HALF = 256                     # token split of the final block


def _build_bass():
    nc = bacc.Bacc("TRN2", target_bir_lowering=False)

    # per-partition byte stream, every segment contiguous [blk][c][i][t]:
    # [b0 | b1 | b2 b3 | b4 b5 | b6 | b7 toks 0:256 | b7 toks 256:512]
    xT = nc.dram_tensor("xT", [128, X_BYTES], FP8, kind="ExternalInput")
    wT = nc.dram_tensor("wT", [128, KC * 2 * 64], FP8, kind="ExternalInput")
    outT = nc.dram_tensor("outT", [M, TOK], F16, kind="ExternalOutput")

    with tile.TileContext(nc) as tc:
        with (
            tc.tile_pool(name="w", bufs=1) as wpool,
            tc.tile_pool(name="x", bufs=1) as xpool,
            tc.tile_pool(name="out", bufs=1) as opool,
            tc.tile_pool(name="sb", bufs=1) as spool,
            tc.tile_pool(name="psum", bufs=NBLK, space="PSUM") as ppool,
        ):
            # w on the scalar ring, ahead of the out DMAs; the sync ring
            # carries only the x stream so its first issue happens ASAP.
            w_tile = wpool.tile([128, KC, 2, 64], FP8)
            nc.scalar.dma_start(
                w_tile[:], wT.rearrange("p (c i m) -> p c i m", c=KC, i=2)
            )

            off = 0

            def fetch(nbytes, tag):
                nonlocal off
                t = xpool.tile([128, nbytes], FP8, tag=tag)
                nc.sync.dma_start(t[:], xT[:, off : off + nbytes])
                off += nbytes
                return t

            # rhs_of[b](c) -> [128, 2, ntok] view;  b = 0..6 full blocks,
            # (7,0)/(7,1) the two token-halves of the last block.
            rhs_of = {}
            for b in (0, 1):  # single blocks first: PE starts early
                t = fetch(KC * 2 * BLK, f"x{b}")
                v = t.rearrange("p (c i t) -> p c i t", c=KC, i=2)
                rhs_of[b] = lambda c, v=v: v[:, c]
            for g, pair in enumerate(((2, 3), (4, 5))):  # 16 KiB runs
                t = fetch(2 * KC * 2 * BLK, f"xg{g}")
                v = t.rearrange("p (b c i t) -> p b c i t", b=2, c=KC, i=2)
                for half in range(2):
                    rhs_of[pair[half]] = lambda c, v=v, half=half: v[:, half, c]
            t6 = fetch(KC * 2 * BLK, "x6")
            v6 = t6.rearrange("p (c i t) -> p c i t", c=KC, i=2)
            rhs_of[6] = lambda c: v6[:, c]
            for h in range(2):  # final block in token halves, 4 KiB runs
                t = fetch(KC * 2 * HALF, f"x7{h}")
                v = t.rearrange("p (c i t) -> p c i t", c=KC, i=2)
                rhs_of[(7, h)] = lambda c, v=v: v[:, c]

            # The matmul codegen supports a single sync wait; this warmup
            # matmul absorbs the w-DMA wait into PE program order so every
            # real matmul needs only its x-DMA wait.
            warm = ppool.tile([64, 64], F32, tag="ptile")
            nc.tensor.matmul(warm[:], w_tile[:, 0], w_tile[:, 0], perf_mode=DR)

            def evac(ptile, cols, out_lo, tag):
                # psumA+psumB -> fp16 on DVE alone (copy then add: back to
                # back on one engine, no cross-engine semaphore on the tail)
                sB = spool.tile([M, cols], F32, tag=f"s{tag}")
                nc.vector.tensor_copy(sB[:], ptile[M : 2 * M, 0:cols])
                o_tile = opool.tile([M, cols], F16, tag=f"o{tag}")
                nc.vector.tensor_add(o_tile[:], ptile[0:M, 0:cols], sB[:])
                nc.scalar.dma_start(outT[:, out_lo : out_lo + cols], o_tile[:])

            for b in range(NBLK - 1):
                ptile = ppool.tile([64, BLK], F32, tag="ptile")
                for c in range(KC):
                    nc.tensor.matmul(
                        ptile[:],
                        w_tile[:, c],
                        rhs_of[b](c),
                        start=(c == 0),
                        stop=(c == KC - 1),
                        perf_mode=DR,
                    )
                evac(ptile, BLK, b * BLK, str(b))

            # final block: two 256-token groups into column halves of one
            # bank; the first half's evac+DMA runs during the second half.
            ptile = ppool.tile([64, BLK], F32, tag="ptile")
            for h in range(2):
                sub = ptile[:, h * HALF : (h + 1) * HALF]
                for c in range(KC):
                    nc.tensor.matmul(
                        sub,
                        w_tile[:, c],
                        rhs_of[(7, h)](c),
                        start=(c == 0),
                        stop=(c == KC - 1),
                        perf_mode=DR,
                    )
                sB = spool.tile([M, HALF], F32, tag=f"s7{h}")
                nc.vector.tensor_copy(sB[:], ptile[M : 2 * M, h * HALF : (h + 1) * HALF])
                o_tile = opool.tile([M, HALF], F16, tag=f"o7{h}")
                nc.vector.tensor_add(
                    o_tile[:], ptile[0:M, h * HALF : (h + 1) * HALF], sB[:]
                )
                nc.scalar.dma_start(
                    outT[:, 7 * BLK + h * HALF : 7 * BLK + (h + 1) * HALF],
                    o_tile[:],
                )

    nc.compile()
    return nc


_NC_CACHE = None


def _get_nc():
    global _NC_CACHE
    if _NC_CACHE is None:
        _NC_CACHE = _build_bass()
    return _NC_CACHE


def _hadamard32() -> np.ndarray:
    h = np.array([[1.0]], dtype=np.float64)
    while h.shape[0] < M:
        h = np.block([[h, h], [h, -h]])
    return h


_NOISE_CACHE = None


def _noise() -> np.ndarray:
    # Mirror reference.py exactly (same op on the default jax backend).
    global _NOISE_CACHE
    if _NOISE_CACHE is None:
        import jax

        nz = NOISE_STD * jax.random.normal(
            jax.random.key(42), (B, N, M), dtype=np.float32
        )
        _NOISE_CACHE = np.asarray(nz)
    return _NOISE_CACHE


def _pack_w(W: np.ndarray) -> np.ndarray:
    """Build the DoubleRow stationary cells [128, KC*2*64] fp8."""
    w_eff = (_hadamard32() @ W.astype(np.float64)) / math.sqrt(M)  # [M, D]
    W16 = 16.0 * w_eff
    whi = W16.astype(np.float32).astype(NP8)
    wlo = (16.0 * (W16 - whi.astype(np.float64))).astype(np.float32).astype(NP8)
    whi_f = whi.astype(np.float32)
    wlo_f = wlo.astype(np.float32)

    cells = np.empty((2, 64, D), dtype=NP8)  # [i, m, d]
    cells[0, 0:M] = whi                       # pairs with xhi
    cells[0, M:] = (wlo_f / 16.0).astype(NP8)
    cells[1, 0:M] = (whi_f / 16.0).astype(NP8)  # pairs with xlo (=16*residual)
    cells[1, M:] = (wlo_f / 256.0).astype(NP8)

    # [i, m, c, p] -> [p, c, i, m]
    wf = cells.reshape(2, 64, KC, 128).transpose(3, 2, 0, 1)
    return np.ascontiguousarray(wf).reshape(128, KC * 2 * 64)


def _pack_x_core(xhi: np.ndarray, xlo: np.ndarray) -> np.ndarray:
    """[TOK, D] hi/lo fp8 -> [128, X_BYTES] per-partition stream."""

    def seg(t0, tn):
        q = np.stack([xhi[t0 : t0 + tn], xlo[t0 : t0 + tn]])  # [2, n, D]
        qr = q.reshape(2, tn, KC, 128)                        # [2, n, c, 128]
        arr = qr.transpose(3, 2, 0, 1)                        # [128, c, 2, n]
        return arr.reshape(128, KC * 2 * tn)

    segs = [seg(b * BLK, BLK) for b in range(7)]
    segs.append(seg(7 * BLK, HALF))
    segs.append(seg(7 * BLK + HALF, HALF))
    return np.ascontiguousarray(np.concatenate(segs, axis=1))


def kernel(x: np.ndarray, W: np.ndarray, _profile_sink=None) -> np.ndarray:
    x = np.ascontiguousarray(np.asarray(x, dtype=np.float32))
    W = np.asarray(W, dtype=np.float32)

    w_dev = _pack_w(W)

    X = x.reshape(TOK_TOTAL, D)
    xhi = X.astype(NP8)
    xlo = (16.0 * (X - xhi.astype(np.float32))).astype(NP8)

    in_maps = []
    for i in range(N_CORES):
        sl = slice(i * TOK, (i + 1) * TOK)
        in_maps.append({"xT": _pack_x_core(xhi[sl], xlo[sl]), "wT": w_dev})

    res = run_bass_kernel_spmd(
        _get_nc(),
        in_maps,
        core_ids=list(range(N_CORES)),
        trace=_profile_sink is not None,
    )
    if _profile_sink is not None:
        _profile_sink.append(res)

    # device result is 16*(x @ w_eff^T), transposed, fp16
    out = np.concatenate(
        [r["outT"].T.astype(np.float32) for r in res.results], axis=0
    )
    out = out.reshape(B, N, M) * (1.0 / 16.0) + _noise()
    return np.ascontiguousarray(out.astype(np.float32))


if __name__ == "__main__":
    xs = np.random.randn(B, N, D).astype(np.float32)
    Ws = (np.random.randn(M, D) / math.sqrt(D)).astype(np.float32)
    o = kernel(xs, Ws)
    print(o.shape, o.dtype)
